# revision 13
# baseline (speedup 1.0000x reference)
"""MultiHeadAttention Trainium2 Bass kernel (8 cores).

Problem: B=2, S=2048, D=1024, H=16 heads, DK=64, fp32.
  q/k/v = x @ W* + b*; scores = q k^T / 8; attn = softmax; ctx = attn v;
  out = ctx @ Wo + bo.

Sharding (8 cores): batch (2-way) x head-group (4-way tensor parallel).
Core c handles b = c // 4 and heads [4g, 4g+4), g = c % 4 (d' slice of 256).
Each core gets x[b]^T and the W column/row slices for its head group, computes
a partial out [S, D] (contraction over its 256 d' rows of Wo), and the host
sums the 4 partials per batch and adds the host-folded bias correction
(bv @ Wo + bo).

On-device layout: "transposed activations". qT/kT [256, S] (d' on
partitions), v natural [S, 256+ones]. Attention per (head, qi-chunk):
  scoresT[kj, qi] = kT^T qT   (PE, fp32r)
  attnT = exp(scoresT / 8)    (ACT, psum->sbuf, fp32r out)
  ctxT[d'+sums, qi] += v_aug^T attnT  (PE; ones col in v gives row sums)
  ctxT /= sums  (partition_broadcast + reciprocal_approx_fast + DVE mul)
out-proj: out[s, :] = sum_mt ctxT[:, mt, s]^T wo[mt]  (PE), DVE drain, DMA.

All matmul operands are float32r (fp32 bits, PE rounds internally; 1 cyc/row
at N>=256 vs 4 cyc/row for exact fp32; measured matmul rel err ~1.5e-4).

The projection matmuls are interleaved into the attention loop (a prologue
computes just enough of kT/qT to start head 0; the rest drips in between
attention steps) so the ACT engine -- the bottleneck (16.8M exps/core) --
starts almost immediately and stays fed.
"""

import numpy as np

B = 2
S = 2048
D = 1024
H = 16
DK = 64
N_CORES = 8
HL = H // 4  # 4 heads per core
DL = HL * DK  # 256 local d'
QC = 1024  # qi chunk for scores/exp
KJT = S // 128  # 16 kj tiles
KT = D // 128  # 8 contraction tiles for projections

_CACHED_NC = None


def _build():
    import concourse.bacc as bacc
    import concourse.mybir as mybir
    import concourse.tile as tile

    f32 = mybir.dt.float32
    f32r = mybir.dt.float32r
    Exp = mybir.ActivationFunctionType.Exp

    nc = bacc.Bacc(None)

    xT = nc.declare_dram_parameter("xT", [D, S], f32r, isOutput=False)
    wq = nc.declare_dram_parameter("wq", [D, DL], f32r, isOutput=False)
    wk = nc.declare_dram_parameter("wk", [D, DL], f32r, isOutput=False)
    wv = nc.declare_dram_parameter("wv", [D, DL], f32r, isOutput=False)
    wo = nc.declare_dram_parameter("wo", [DL, D], f32r, isOutput=False)
    bq = nc.declare_dram_parameter("bq", [128, 2], f32, isOutput=False)
    bk = nc.declare_dram_parameter("bk", [128, 2], f32, isOutput=False)
    out = nc.declare_dram_parameter("out", [S, D], f32, isOutput=True)

    with tile.TileContext(nc) as tc:
        with (
            tc.tile_pool(name="persist", bufs=1) as persist,
            tc.tile_pool(name="ph1", bufs=1) as ph1,
            tc.tile_pool(name="attn", bufs=4) as atp,
            tc.tile_pool(name="norm", bufs=2) as npl,
            tc.tile_pool(name="ob", bufs=2) as obp,
            tc.tile_pool(name="scps", bufs=3, space="PSUM") as scp,
            tc.tile_pool(name="cxps", bufs=2, space="PSUM") as cxp,
        ):
            qT_sb = persist.tile([128, 2, S], f32r, tag="qT")
            kT_sb = persist.tile([128, 2, S], f32r, tag="kT")
            v_sb = persist.tile([128, KJT, HL, DK + 1], f32r, tag="v")
            ctxT_sb = persist.tile([128, 2, S], f32r, tag="ctxT")
            wo_sb = persist.tile([128, 2, D], f32r, tag="wo")
            bq_sb = persist.tile([128, 2], f32, tag="bq")
            bk_sb = persist.tile([128, 2], f32, tag="bk")
            ones_f32 = persist.tile([128, KJT, HL, 1], f32, tag="ones")

            nc.sync.dma_start(out=bq_sb[:], in_=bq[:])
            nc.sync.dma_start(out=bk_sb[:], in_=bk[:])
            for mt in range(2):
                nc.sync.dma_start(
                    out=wo_sb[:, mt, :], in_=wo[mt * 128 : (mt + 1) * 128, :]
                )
            nc.vector.memset(ones_f32[:], 1.0)
            nc.vector.tensor_copy(v_sb[:, :, :, DK : DK + 1], ones_f32[:])

            xt, wq_t, wk_t, wv_t = [], [], [], []
            for kt in range(KT):
                t = ph1.tile([128, S], f32r, tag=f"xt{kt}")
                nc.sync.dma_start(out=t[:], in_=xT[kt * 128 : (kt + 1) * 128, :])
                xt.append(t)
                for nm, lst, prm in (
                    ("wq", wq_t, wq),
                    ("wk", wk_t, wk),
                    ("wv", wv_t, wv),
                ):
                    w = ph1.tile([128, DL], f32r, tag=f"{nm}{kt}")
                    nc.sync.dma_start(
                        out=w[:], in_=prm[kt * 128 : (kt + 1) * 128, :]
                    )
                    lst.append(w)

            def qk_chunk(which, mt, n):
                """Project one [128, 512] chunk of qT (which=0) / kT (which=1)."""
                wt, dst, bias = (
                    (wq_t, qT_sb, bq_sb) if which == 0 else (wk_t, kT_sb, bk_sb)
                )
                ns = slice(n * 512, (n + 1) * 512)
                ps = scp.tile([128, 512], f32, tag="sc", name=f"pj{which}{mt}{n}")
                for kt in range(KT):
                    nc.tensor.matmul(
                        ps[:],
                        wt[kt][:, mt * 128 : (mt + 1) * 128],
                        xt[kt][:, ns],
                        start=(kt == 0),
                        stop=(kt == KT - 1),
                    )
                nc.vector.tensor_scalar_add(
                    out=dst[:, mt, ns], in0=ps[:], scalar1=bias[:, mt : mt + 1]
                )

            def v_chunk(jt):
                """Project v rows [jt*128, (jt+1)*128) for all 4 heads."""
                js = slice(jt * 128, (jt + 1) * 128)
                ps = scp.tile([128, DL], f32, tag="sc", name=f"vp{jt}")
                for kt in range(KT):
                    nc.tensor.matmul(
                        ps[:],
                        xt[kt][:, js],
                        wv_t[kt][:],
                        start=(kt == 0),
                        stop=(kt == KT - 1),
                    )
                nc.vector.tensor_copy(
                    v_sb[:, jt, :, 0:DK],
                    ps[:].rearrange("p (h d) -> p h d", h=HL),
                )

            def out_proj_piece(st, nt, c, tail=False):
                s0 = c * 512 + st * 128
                op = scp.tile([128, 512], f32, tag="sc", name=f"op{c}{st}{nt}")
                for mt2 in range(2):
                    nc.tensor.matmul(
                        op[:],
                        ctxT_sb[:, mt2, s0 : s0 + 128],
                        wo_sb[:, mt2, nt * 512 : (nt + 1) * 512],
                        start=(mt2 == 0),
                        stop=(mt2 == 1),
                    )
                ob = obp.tile([128, 512], f32, tag="ob")
                if tail and (st + nt) % 2 == 0:
                    nc.scalar.copy(ob[:], op[:])  # ACT is idle in the tail
                else:
                    nc.vector.tensor_copy(ob[:], op[:])
                nc.sync.dma_start(
                    out=out[s0 : s0 + 128, nt * 512 : (nt + 1) * 512],
                    in_=ob[:],
                )

            # Prologue: just enough for chunk-phase (c=0, mt=0) to start.
            qk_chunk(1, 0, 0)  # kT mt0 n0 (kj tiles 0-3)
            qk_chunk(0, 0, 0)  # qT mt0 n0 (first 512 qi)

            # Remaining work dripped into the attention loops, emitted between
            # a step's exp and its ctx matmuls so the PE work hides in the
            # exp's shadow. Phases are (c, mt) pairs in order:
            #   (0,0) (0,1) (1,0) (1,1) (2,0) (2,1) (3,0) (3,1)
            # mid[(c, mt, kj)] = list of thunks.
            mid = {}
            # (0,0): rest of kT mt0 (needed by its own kj>=4), the (0,1)
            # phase's kT mt1 n0 + qT mt1 n0, and all v chunks (jt=kj).
            mid[(0, 0, 1)] = [lambda: qk_chunk(1, 0, 1)]
            mid[(0, 0, 3)] = [lambda: qk_chunk(1, 0, 2)]
            mid[(0, 0, 5)] = [lambda: qk_chunk(1, 0, 3)]
            mid[(0, 0, 7)] = [lambda: qk_chunk(1, 1, 0)]
            mid[(0, 0, 9)] = [lambda: qk_chunk(0, 1, 0)]
            # (0,1): rest of kT mt1; qT chunks for phase (1,*)
            mid[(0, 1, 1)] = [lambda: qk_chunk(1, 1, 1)]
            mid[(0, 1, 3)] = [lambda: qk_chunk(1, 1, 2)]
            mid[(0, 1, 5)] = [lambda: qk_chunk(1, 1, 3)]
            mid[(0, 1, 7)] = [lambda: qk_chunk(0, 0, 1)]
            mid[(0, 1, 9)] = [lambda: qk_chunk(0, 1, 1)]
            # later qT chunks, one phase ahead of use
            mid[(1, 0, 1)] = [lambda: qk_chunk(0, 0, 2)]
            mid[(1, 1, 1)] = [lambda: qk_chunk(0, 1, 2)]
            mid[(2, 0, 1)] = [lambda: qk_chunk(0, 0, 3)]
            mid[(2, 1, 1)] = [lambda: qk_chunk(0, 1, 3)]
            # out-proj for chunk c drips into chunk c+1's mt=0 phase, keeping
            # mt=1 phases as pure exp streams (cleaner tail).
            for c in range(3):
                for i in range(8):
                    st, nt = i // 2, i % 2
                    mid.setdefault((c + 1, 0, 1 + 2 * i), []).append(
                        lambda st=st, nt=nt, c=c: out_proj_piece(st, nt, c)
                    )

            NCH = S // 512  # 4 qi chunks of 512
            steps = [
                (c, mt, kj)
                for c in range(NCH)
                for mt in range(2)
                for kj in range(KJT)
            ]
            sc_t = {}

            def emit_sc(i):
                c, mt, kj = steps[i]
                sc = scp.tile([128, QC], f32, tag="sc", name=f"sc{c}{mt}{kj}")
                col = slice(c * 512, (c + 1) * 512)
                for hp in range(2):
                    hs = slice(64 * hp, 64 * hp + 64)
                    nc.tensor.matmul(
                        sc[:, hp * 512 : (hp + 1) * 512],
                        kT_sb[hs, mt, kj * 128 : (kj + 1) * 128],
                        qT_sb[hs, mt, col],
                        start=True,
                        stop=True,
                    )
                sc_t[i] = sc

            cxh = {}
            emit_sc(0)
            emit_sc(1)
            for i, (c, mt, kj) in enumerate(steps):
                col = slice(c * 512, (c + 1) * 512)
                if kj == 0:
                    cxh[(c, mt)] = [
                        cxp.tile(
                            [DK + 1, 512], f32, tag="cx", name=f"cx{c}{mt}{j}"
                        )
                        for j in range(2)
                    ]
                at = atp.tile([128, QC], f32r, tag="at")
                nc.scalar.activation(at[:], sc_t.pop(i)[:], Exp, scale=0.125)
                # scores two steps ahead, then background work, then ctx --
                # keeps the next exp's input first in PE program order so the
                # dripped matmuls hide in the exp shadow.
                if i + 2 < len(steps):
                    emit_sc(i + 2)
                for th in mid.get((c, mt, kj), ()):
                    th()
                if c == 0 and mt == 0:
                    v_chunk(kj)
                for hp in range(2):
                    nc.tensor.matmul(
                        cxh[(c, mt)][hp][:],
                        v_sb[:, kj, 2 * mt + hp, :],
                        at[:, hp * 512 : (hp + 1) * 512],
                        start=(kj == 0),
                        stop=(kj == KJT - 1),
                    )
                if kj == KJT - 1:
                    # normalize both heads: ctxT = cx[0:64] / cx[64].
                    # Copy psum->sbuf immediately (frees the psum bank), then
                    # run the whole chain in SBUF.
                    for hp in range(2):
                        cx = cxh[(c, mt)][hp]
                        cxs = npl.tile([DK + 1, 512], f32, tag="cxs")
                        nc.vector.tensor_copy(cxs[:], cx[:])
                        srow0 = npl.tile([1, 512], f32, tag="srow0")
                        nc.sync.dma_start(
                            out=srow0[:], in_=cxs[DK : DK + 1, :]
                        )
                        sbc = npl.tile([64, 512], f32, tag="sbc")
                        nc.gpsimd.partition_broadcast(sbc[:], srow0[:])
                        rinv = npl.tile([64, 512], f32, tag="rinv")
                        nc.vector.reciprocal_approx_fast(
                            out=rinv[:], in_=sbc[:]
                        )
                        if hp == 0:
                            nc.vector.tensor_mul(
                                ctxT_sb[0:64, mt, col], cxs[0:64, :], rinv[:]
                            )
                        else:
                            tmp = npl.tile([64, 512], f32r, tag="sbc")
                            nc.vector.tensor_mul(tmp[:], cxs[0:64, :], rinv[:])
                            nc.sync.dma_start(
                                out=ctxT_sb[64:128, mt, col], in_=tmp[:]
                            )
            # last chunk's out-proj is the unavoidable tail
            for st in range(4):
                for nt in range(2):
                    out_proj_piece(st, nt, NCH - 1, tail=True)

    nc.compile()
    return nc


def _get_nc():
    global _CACHED_NC
    if _CACHED_NC is None:
        _CACHED_NC = _build()
    return _CACHED_NC


def _in_maps(x, Wq, bq, Wk, bk, Wv, bv, Wo, bo):
    xTs = [np.ascontiguousarray(x[b].T) for b in range(B)]
    maps = []
    for c in range(N_CORES):
        b, g = c // 4, c % 4
        cs = slice(g * DL, (g + 1) * DL)
        maps.append(
            {
                "xT": xTs[b],
                "wq": np.ascontiguousarray(Wq[:, cs]),
                "wk": np.ascontiguousarray(Wk[:, cs]),
                "wv": np.ascontiguousarray(Wv[:, cs]),
                "wo": np.ascontiguousarray(Wo[cs, :]),
                "bq": np.ascontiguousarray(bq[cs].reshape(2, 128).T),
                "bk": np.ascontiguousarray(bk[cs].reshape(2, 128).T),
            }
        )
    return maps


def _assemble(results, bv, Wo, bo):
    corr = (bv.astype(np.float64) @ Wo.astype(np.float64)) + bo.astype(np.float64)
    outs = []
    for b in range(B):
        acc = np.zeros((S, D), dtype=np.float64)
        for g in range(4):
            acc += results[b * 4 + g]["out"].astype(np.float64)
        outs.append((acc + corr).astype(np.float32))
    return np.stack(outs)


def kernel(x, Wq, bq, Wk, bk, Wv, bv, Wo, bo):
    from concourse.bass_utils import run_bass_kernel_spmd

    x = np.asarray(x, dtype=np.float32)
    Wq = np.asarray(Wq, dtype=np.float32)
    Wk = np.asarray(Wk, dtype=np.float32)
    Wv = np.asarray(Wv, dtype=np.float32)
    Wo = np.asarray(Wo, dtype=np.float32)
    bq = np.asarray(bq, dtype=np.float32)
    bk = np.asarray(bk, dtype=np.float32)
    bv = np.asarray(bv, dtype=np.float32)
    bo = np.asarray(bo, dtype=np.float32)

    nc = _get_nc()
    res = run_bass_kernel_spmd(
        nc, _in_maps(x, Wq, bq, Wk, bk, Wv, bv, Wo, bo), core_ids=list(range(N_CORES))
    )
    return _assemble(res.results, bv, Wo, bo)


# revision 14
# speedup vs baseline: 1.0322x; 1.0322x over previous
"""MultiHeadAttention Trainium2 Bass kernel (8 cores).

Problem: B=2, S=2048, D=1024, H=16 heads, DK=64, fp32.
  q/k/v = x @ W* + b*; scores = q k^T / 8; attn = softmax; ctx = attn v;
  out = ctx @ Wo + bo.

Sharding (8 cores): batch (2-way) x head-group (4-way tensor parallel).
Core c handles b = c // 4 and heads [4g, 4g+4), g = c % 4 (d' slice of 256).
Each core gets x[b]^T and the W column/row slices for its head group, computes
a partial out [S, D] (contraction over its 256 d' rows of Wo), and the host
sums the 4 partials per batch and adds the host-folded bias correction
(bv @ Wo + bo).

On-device layout: "transposed activations". qT/kT [256, S] (d' on
partitions), v natural [S, 256+ones]. Attention per (head, qi-chunk):
  scoresT[kj, qi] = kT^T qT   (PE, fp32r)
  attnT = exp(scoresT / 8)    (ACT, psum->sbuf, fp32r out)
  ctxT[d'+sums, qi] += v_aug^T attnT  (PE; ones col in v gives row sums)
  ctxT /= sums  (partition_broadcast + reciprocal_approx_fast + DVE mul)
out-proj: out[s, :] = sum_mt ctxT[:, mt, s]^T wo[mt]  (PE), DVE drain, DMA.

All matmul operands are float32r (fp32 bits, PE rounds internally; 1 cyc/row
at N>=256 vs 4 cyc/row for exact fp32; measured matmul rel err ~1.5e-4).

The projection matmuls are interleaved into the attention loop (a prologue
computes just enough of kT/qT to start head 0; the rest drips in between
attention steps) so the ACT engine -- the bottleneck (16.8M exps/core) --
starts almost immediately and stays fed.
"""

import numpy as np

B = 2
S = 2048
D = 1024
H = 16
DK = 64
N_CORES = 8
HL = H // 4  # 4 heads per core
DL = HL * DK  # 256 local d'
QC = 1024  # qi chunk for scores/exp
KJT = S // 128  # 16 kj tiles
KT = D // 128  # 8 contraction tiles for projections

_CACHED_NC = None


def _build():
    import concourse.bacc as bacc
    import concourse.mybir as mybir
    import concourse.tile as tile

    f32 = mybir.dt.float32
    f32r = mybir.dt.float32r
    Exp = mybir.ActivationFunctionType.Exp

    nc = bacc.Bacc(None)

    xT = nc.declare_dram_parameter("xT", [D, S], f32r, isOutput=False)
    wq = nc.declare_dram_parameter("wq", [D, DL], f32r, isOutput=False)
    wk = nc.declare_dram_parameter("wk", [D, DL], f32r, isOutput=False)
    wv = nc.declare_dram_parameter("wv", [D, DL], f32r, isOutput=False)
    wo = nc.declare_dram_parameter("wo", [DL, D], f32r, isOutput=False)
    bq = nc.declare_dram_parameter("bq", [128, 2], f32, isOutput=False)
    bk = nc.declare_dram_parameter("bk", [128, 2], f32, isOutput=False)
    out = nc.declare_dram_parameter("out", [S, D], f32, isOutput=True)

    with tile.TileContext(nc) as tc:
        with (
            tc.tile_pool(name="persist", bufs=1) as persist,
            tc.tile_pool(name="ph1", bufs=1) as ph1,
            tc.tile_pool(name="attn", bufs=4) as atp,
            tc.tile_pool(name="norm", bufs=2) as npl,
            tc.tile_pool(name="ob", bufs=2) as obp,
            tc.tile_pool(name="scps", bufs=3, space="PSUM") as scp,
            tc.tile_pool(name="cxps", bufs=2, space="PSUM") as cxp,
        ):
            qT_sb = persist.tile([128, 2, S], f32r, tag="qT")
            kT_sb = persist.tile([128, 2, S], f32r, tag="kT")
            v_sb = persist.tile([128, KJT, HL, DK + 1], f32r, tag="v")
            ctxT_sb = persist.tile([128, 2, S], f32r, tag="ctxT")
            wo_sb = persist.tile([128, 2, D], f32r, tag="wo")
            bq_sb = persist.tile([128, 2], f32, tag="bq")
            bk_sb = persist.tile([128, 2], f32, tag="bk")
            ones_f32 = persist.tile([128, KJT, HL, 1], f32, tag="ones")

            nc.sync.dma_start(out=bq_sb[:], in_=bq[:])
            nc.sync.dma_start(out=bk_sb[:], in_=bk[:])
            for mt in range(2):
                nc.sync.dma_start(
                    out=wo_sb[:, mt, :], in_=wo[mt * 128 : (mt + 1) * 128, :]
                )
            nc.vector.memset(ones_f32[:], 1.0)
            nc.vector.tensor_copy(v_sb[:, :, :, DK : DK + 1], ones_f32[:])

            xt, wq_t, wk_t, wv_t = [], [], [], []
            for kt in range(KT):
                t = ph1.tile([128, S], f32r, tag=f"xt{kt}")
                nc.sync.dma_start(out=t[:], in_=xT[kt * 128 : (kt + 1) * 128, :])
                xt.append(t)
                for nm, lst, prm in (
                    ("wq", wq_t, wq),
                    ("wk", wk_t, wk),
                    ("wv", wv_t, wv),
                ):
                    w = ph1.tile([128, DL], f32r, tag=f"{nm}{kt}")
                    nc.sync.dma_start(
                        out=w[:], in_=prm[kt * 128 : (kt + 1) * 128, :]
                    )
                    lst.append(w)

            def qk_chunk(which, mt, n):
                """Project one [128, 512] chunk of qT (which=0) / kT (which=1)."""
                wt, dst, bias = (
                    (wq_t, qT_sb, bq_sb) if which == 0 else (wk_t, kT_sb, bk_sb)
                )
                ns = slice(n * 512, (n + 1) * 512)
                ps = scp.tile([128, 512], f32, tag="sc", name=f"pj{which}{mt}{n}")
                for kt in range(KT):
                    nc.tensor.matmul(
                        ps[:],
                        wt[kt][:, mt * 128 : (mt + 1) * 128],
                        xt[kt][:, ns],
                        start=(kt == 0),
                        stop=(kt == KT - 1),
                    )
                nc.vector.tensor_scalar_add(
                    out=dst[:, mt, ns], in0=ps[:], scalar1=bias[:, mt : mt + 1]
                )

            def v_chunk(jt):
                """Project v rows [jt*128, (jt+1)*128) for all 4 heads."""
                js = slice(jt * 128, (jt + 1) * 128)
                ps = scp.tile([128, DL], f32, tag="sc", name=f"vp{jt}")
                for kt in range(KT):
                    nc.tensor.matmul(
                        ps[:],
                        xt[kt][:, js],
                        wv_t[kt][:],
                        start=(kt == 0),
                        stop=(kt == KT - 1),
                    )
                nc.vector.tensor_copy(
                    v_sb[:, jt, :, 0:DK],
                    ps[:].rearrange("p (h d) -> p h d", h=HL),
                )

            def out_proj_piece(st, nt, c, tail=False):
                s0 = c * 512 + st * 128
                op = scp.tile([128, 512], f32, tag="sc", name=f"op{c}{st}{nt}")
                for mt2 in range(2):
                    nc.tensor.matmul(
                        op[:],
                        ctxT_sb[:, mt2, s0 : s0 + 128],
                        wo_sb[:, mt2, nt * 512 : (nt + 1) * 512],
                        start=(mt2 == 0),
                        stop=(mt2 == 1),
                    )
                ob = obp.tile([128, 512], f32, tag="ob")
                if tail and (st + nt) % 2 == 0:
                    nc.scalar.copy(ob[:], op[:])  # ACT is idle in the tail
                else:
                    nc.vector.tensor_copy(ob[:], op[:])
                nc.sync.dma_start(
                    out=out[s0 : s0 + 128, nt * 512 : (nt + 1) * 512],
                    in_=ob[:],
                )

            # Prologue: just enough for chunk-phase (c=0, mt=0) to start.
            qk_chunk(1, 0, 0)  # kT mt0 n0 (kj tiles 0-3)
            qk_chunk(0, 0, 0)  # qT mt0 n0 (first 512 qi)

            # Remaining work dripped into the attention loops, emitted between
            # a step's exp and its ctx matmuls so the PE work hides in the
            # exp's shadow. Phases are (c, mt) pairs in order:
            #   (0,0) (0,1) (1,0) (1,1) (2,0) (2,1) (3,0) (3,1)
            # mid[(c, mt, kj)] = list of thunks.
            mid = {}
            # (0,0): rest of kT mt0 (needed by its own kj>=4), the (0,1)
            # phase's kT mt1 n0 + qT mt1 n0, and all v chunks (jt=kj).
            mid[(0, 0, 1)] = [lambda: qk_chunk(1, 0, 1)]
            mid[(0, 0, 3)] = [lambda: qk_chunk(1, 0, 2)]
            mid[(0, 0, 5)] = [lambda: qk_chunk(1, 0, 3)]
            mid[(0, 0, 7)] = [lambda: qk_chunk(1, 1, 0)]
            mid[(0, 0, 9)] = [lambda: qk_chunk(0, 1, 0)]
            # (0,1): rest of kT mt1; qT chunks for phase (1,*)
            mid[(0, 1, 1)] = [lambda: qk_chunk(1, 1, 1)]
            mid[(0, 1, 3)] = [lambda: qk_chunk(1, 1, 2)]
            mid[(0, 1, 5)] = [lambda: qk_chunk(1, 1, 3)]
            mid[(0, 1, 7)] = [lambda: qk_chunk(0, 0, 1)]
            mid[(0, 1, 9)] = [lambda: qk_chunk(0, 1, 1)]
            # later qT chunks, one phase ahead of use
            mid[(1, 0, 1)] = [lambda: qk_chunk(0, 0, 2)]
            mid[(1, 1, 1)] = [lambda: qk_chunk(0, 1, 2)]
            mid[(2, 0, 1)] = [lambda: qk_chunk(0, 0, 3)]
            mid[(2, 1, 1)] = [lambda: qk_chunk(0, 1, 3)]
            # out-proj for chunk c drips into chunk c+1's phases
            for c in range(3):
                for i in range(8):
                    st, nt = i // 2, i % 2
                    mt_, kj_ = (0, 3 + 2 * (i % 4)) if i < 4 else (1, 3 + 2 * (i % 4))
                    mid.setdefault((c + 1, mt_, kj_), []).append(
                        lambda st=st, nt=nt, c=c: out_proj_piece(st, nt, c)
                    )

            NCH = S // 512  # 4 qi chunks of 512
            steps = [
                (c, mt, kj)
                for c in range(NCH)
                for mt in range(2)
                for kj in range(KJT)
            ]
            sc_t = {}

            def emit_sc(i):
                c, mt, kj = steps[i]
                sc = scp.tile([128, QC], f32, tag="sc", name=f"sc{c}{mt}{kj}")
                col = slice(c * 512, (c + 1) * 512)
                for hp in range(2):
                    hs = slice(64 * hp, 64 * hp + 64)
                    nc.tensor.matmul(
                        sc[:, hp * 512 : (hp + 1) * 512],
                        kT_sb[hs, mt, kj * 128 : (kj + 1) * 128],
                        qT_sb[hs, mt, col],
                        start=True,
                        stop=True,
                    )
                sc_t[i] = sc

            cxh = {}
            emit_sc(0)
            emit_sc(1)
            for i, (c, mt, kj) in enumerate(steps):
                col = slice(c * 512, (c + 1) * 512)
                if kj == 0:
                    cxh[(c, mt)] = [
                        cxp.tile(
                            [DK + 1, 512], f32, tag="cx", name=f"cx{c}{mt}{j}"
                        )
                        for j in range(2)
                    ]
                at = atp.tile([128, QC], f32r, tag="at")
                nc.scalar.activation(at[:], sc_t.pop(i)[:], Exp, scale=0.125)
                # scores two steps ahead, then background work, then ctx --
                # keeps the next exp's input first in PE program order so the
                # dripped matmuls hide in the exp shadow.
                if i + 2 < len(steps):
                    emit_sc(i + 2)
                for th in mid.get((c, mt, kj), ()):
                    th()
                if c == 0 and mt == 0:
                    v_chunk(kj)
                for hp in range(2):
                    nc.tensor.matmul(
                        cxh[(c, mt)][hp][:],
                        v_sb[:, kj, 2 * mt + hp, :],
                        at[:, hp * 512 : (hp + 1) * 512],
                        start=(kj == 0),
                        stop=(kj == KJT - 1),
                    )
                if kj == KJT - 1:
                    # normalize both heads: ctxT = cx[0:64] / cx[64].
                    # Copy psum->sbuf immediately (frees the psum bank), then
                    # run the whole chain in SBUF.
                    for hp in range(2):
                        cx = cxh[(c, mt)][hp]
                        cxs = npl.tile([DK + 1, 512], f32, tag="cxs")
                        nc.vector.tensor_copy(cxs[:], cx[:])
                        srow0 = npl.tile([1, 512], f32, tag="srow0")
                        nc.sync.dma_start(
                            out=srow0[:], in_=cxs[DK : DK + 1, :]
                        )
                        sbc = npl.tile([64, 512], f32, tag="sbc")
                        nc.gpsimd.partition_broadcast(sbc[:], srow0[:])
                        rinv = npl.tile([64, 512], f32, tag="rinv")
                        nc.vector.reciprocal_approx_fast(
                            out=rinv[:], in_=sbc[:]
                        )
                        if hp == 0:
                            nc.vector.tensor_mul(
                                ctxT_sb[0:64, mt, col], cxs[0:64, :], rinv[:]
                            )
                        else:
                            tmp = npl.tile([64, 512], f32r, tag="sbc")
                            nc.vector.tensor_mul(tmp[:], cxs[0:64, :], rinv[:])
                            nc.sync.dma_start(
                                out=ctxT_sb[64:128, mt, col], in_=tmp[:]
                            )
            # last chunk's out-proj is the unavoidable tail
            for st in range(4):
                for nt in range(2):
                    out_proj_piece(st, nt, NCH - 1, tail=True)

    nc.compile()
    return nc


def _get_nc():
    global _CACHED_NC
    if _CACHED_NC is None:
        _CACHED_NC = _build()
    return _CACHED_NC


def _in_maps(x, Wq, bq, Wk, bk, Wv, bv, Wo, bo):
    xTs = [np.ascontiguousarray(x[b].T) for b in range(B)]
    maps = []
    for c in range(N_CORES):
        b, g = c // 4, c % 4
        cs = slice(g * DL, (g + 1) * DL)
        maps.append(
            {
                "xT": xTs[b],
                "wq": np.ascontiguousarray(Wq[:, cs]),
                "wk": np.ascontiguousarray(Wk[:, cs]),
                "wv": np.ascontiguousarray(Wv[:, cs]),
                "wo": np.ascontiguousarray(Wo[cs, :]),
                "bq": np.ascontiguousarray(bq[cs].reshape(2, 128).T),
                "bk": np.ascontiguousarray(bk[cs].reshape(2, 128).T),
            }
        )
    return maps


def _assemble(results, bv, Wo, bo):
    corr = (bv.astype(np.float64) @ Wo.astype(np.float64)) + bo.astype(np.float64)
    outs = []
    for b in range(B):
        acc = np.zeros((S, D), dtype=np.float64)
        for g in range(4):
            acc += results[b * 4 + g]["out"].astype(np.float64)
        outs.append((acc + corr).astype(np.float32))
    return np.stack(outs)


def kernel(x, Wq, bq, Wk, bk, Wv, bv, Wo, bo):
    from concourse.bass_utils import run_bass_kernel_spmd

    x = np.asarray(x, dtype=np.float32)
    Wq = np.asarray(Wq, dtype=np.float32)
    Wk = np.asarray(Wk, dtype=np.float32)
    Wv = np.asarray(Wv, dtype=np.float32)
    Wo = np.asarray(Wo, dtype=np.float32)
    bq = np.asarray(bq, dtype=np.float32)
    bk = np.asarray(bk, dtype=np.float32)
    bv = np.asarray(bv, dtype=np.float32)
    bo = np.asarray(bo, dtype=np.float32)

    nc = _get_nc()
    res = run_bass_kernel_spmd(
        nc, _in_maps(x, Wq, bq, Wk, bk, Wv, bv, Wo, bo), core_ids=list(range(N_CORES))
    )
    return _assemble(res.results, bv, Wo, bo)
